# revision 27
# baseline (speedup 1.0000x reference)
"""Causal self-attention on 8 TRN2 NeuronCores (Bass/Tile, SPMD).

Problem: y = CausalSelfAttention(x; Wqkv, bqkv, Wproj, bproj)
  x [B=4, T=2048, C=1024], H=16 heads, D=64.

Sharding: core c = (batch b = c//2, head-half hh = c%2). Each core computes
q/k/v for its 8 heads of its batch (Wqkv column-sharded), full causal
attention for those heads, and a partial output projection (Wproj
row-sharded). Host sums the two partials per batch and adds bproj.

Per-core kernel (all matmuls bf16 with fp32 PSUM accumulation):
  - q,k are produced d-major ([CL, T]) so QK^T needs no transposes;
    scores come out k-major [128 k, 512 q] per tile.
  - softmax skips the max-subtraction (scores are O(1) here; exp is safe)
    so it is a single fused exp on the Scalar engine; the causal mask is
    a bf16 multiply on the diagonal blocks only. Row sums come free from
    an extra ones-column appended to each per-head V tile (M=65 AV
    matmul), and 1/sum is broadcast across partitions via a tiny
    DRAM round-trip DMA.
  - Sub-diagonal k-tiles are skipped entirely (half the attention work).
"""

import math
from contextlib import ExitStack

import numpy as np
import ml_dtypes

import concourse.tile as tile
from concourse import bacc, mybir

BF16 = mybir.dt.bfloat16
F32 = mybir.dt.float32
NPBF16 = ml_dtypes.bfloat16

P = 128  # partitions / k-tile size
QB = 512  # q-block (matmul N; one fp32 PSUM bank)

B, T, C, H, D = 4, 2048, 1024, 16, 64
N_CORES = 8
HL = H // (N_CORES // B)  # heads per core (8)
CL = HL * D  # local head width (512)

# ---------------------------------------------------------------------------
# Per-core Bass program
# ---------------------------------------------------------------------------


def build_kernel(T=T, C=C, HL=HL, D=D, Cout=C):
    CL = HL * D
    n_ct = C // P
    n_mt = CL // P
    n_tt = T // P
    n_qb = T // QB
    n_hp = HL // 2
    dpb = QB // P
    n_cb = Cout // QB
    scale = 1.0 / math.sqrt(D)
    D1 = D + 1
    n_sums = n_hp * n_qb * 2  # one softmax-denominator row per (head, q-block)

    assert C % P == 0 and CL % P == 0 and T % QB == 0 and Cout % QB == 0
    assert HL % 2 == 0 and D == 64 and n_mt == n_hp and n_sums <= P

    nc = bacc.Bacc("TRN2", target_bir_lowering=False, debug=False)
    xT = nc.dram_tensor("xT", [C, T], BF16, kind="ExternalInput")
    wq = nc.dram_tensor("wq", [C, CL], BF16, kind="ExternalInput")
    wk = nc.dram_tensor("wk", [C, CL], BF16, kind="ExternalInput")
    wv = nc.dram_tensor("wv", [C, CL], BF16, kind="ExternalInput")
    wp = nc.dram_tensor("wp", [CL, Cout], BF16, kind="ExternalInput")
    masks = nc.dram_tensor("masks", [P, P], BF16, kind="ExternalInput")
    out = nc.dram_tensor("out", [T, Cout], F32, kind="ExternalOutput")

    with tile.TileContext(nc) as tc, ExitStack() as ctx:
        persist = ctx.enter_context(tc.tile_pool(name="persist", bufs=1))
        # PSUM budget (8 banks): u512 4 x [128,512] + st2 2 x [128,1024]
        ps_u512 = ctx.enter_context(tc.tile_pool(name="ps_u512", bufs=4, space="PSUM"))
        ps_st2 = ctx.enter_context(tc.tile_pool(name="ps_st2", bufs=2, space="PSUM"))
        ppool = ctx.enter_context(tc.tile_pool(name="ppool", bufs=6))
        spool = ctx.enter_context(tc.tile_pool(name="spool", bufs=4))
        bcpool = ctx.enter_context(tc.tile_pool(name="bcpool", bufs=4))
        stage = ctx.enter_context(tc.tile_pool(name="stage", bufs=4))
        dram = ctx.enter_context(tc.tile_pool(name="dram", bufs=1, space="DRAM"))

        # ---- persistent loads (v needs xT+wv first; wq/wk next; wp last) ----
        def load_tiles(src, n, rows, cols, tagp):
            ts = []
            for i in range(n):
                t = persist.tile([rows, cols], BF16, tag=f"{tagp}{i}", name=f"{tagp}{i}")
                nc.sync.dma_start(t[:], src[i * rows : (i + 1) * rows, :])
                ts.append(t)
            return ts

        xT_sb, wv_sb = [], []
        for i in range(n_ct):
            t = persist.tile([P, T], BF16, tag=f"xT{i}", name=f"xT{i}")
            nc.sync.dma_start(t[:], xT[i * P : (i + 1) * P, :])
            xT_sb.append(t)
            t = persist.tile([P, CL], BF16, tag=f"wv{i}", name=f"wv{i}")
            nc.sync.dma_start(t[:], wv[i * P : (i + 1) * P, :])
            wv_sb.append(t)
        trimask = persist.tile([P, P], BF16, tag="trimask", name="trimask")
        nc.sync.dma_start(trimask[:], masks[:])
        wq_sb = load_tiles(wq, n_ct, P, CL, "wq")
        wk_sb = load_tiles(wk, n_ct, P, CL, "wk")
        wp_sb = load_tiles(wp, n_mt, P, Cout, "wp")

        sums_d = dram.tile([n_sums, QB], F32, tag="sums_d", name="sums_d")
        recips_d = dram.tile([n_sums, QB], F32, tag="recips_d", name="recips_d")

        # ---- v: interleaved ones column per head: v1 [T, HL*(D+1)] ----
        # Emitted in two chunks: attention on q-block 0 only needs the first
        # dpb v-tiles, so the rest of v overlaps the (ACT-paced) attention.
        v1_sb = [
            persist.tile([P, HL * D1], BF16, tag=f"v1_{tt}", name=f"v1_{tt}")
            for tt in range(n_tt)
        ]

        def emit_v(tts):
            for tt in tts:
                t = v1_sb[tt]
                ones_view = t[:].rearrange("p (h e) -> p h e", h=HL)[:, :, D : D + 1]
                nc.vector.memset(ones_view, 1.0)
                ps = ps_u512.tile([P, CL], F32, tag="u512", name="u512")
                for c in range(n_ct):
                    nc.tensor.matmul(
                        ps[:],
                        xT_sb[c][:, tt * P : (tt + 1) * P],
                        wv_sb[c][:],
                        start=(c == 0),
                        stop=(c == n_ct - 1),
                    )
                dst_view = t[:].rearrange("p (h e) -> p h e", h=HL)[:, :, 0:D]
                src_view = ps[:].rearrange("p (h e) -> p h e", h=HL)
                nc.vector.tensor_copy(dst_view, src_view)

        emit_v(range(min(dpb, n_tt)))

        # ---- per head-pair: q/k projections then attention ----
        yT_sb = [
            persist.tile([P, T], BF16, tag=f"yT{m}", name=f"yT{m}")
            for m in range(n_mt)
        ]
        q_d = [None] * n_mt
        k_d = [None] * n_mt

        def sum_row(hp, qb, i):
            return (hp * n_qb + qb) * 2 + i

        def emit_qk(hp):
            # q_d[hp], k_d[hp]: d-major, one [128, QB] tile per q-block so
            # attention on q-block b can start as soon as block b is cast
            # (head 2*hp rows 0:64, head 2*hp+1 rows 64:128)
            q_d[hp] = []
            k_d[hp] = []
            for b in range(n_qb):
                for name, w_sb, dst in (("k", wk_sb, k_d), ("q", wq_sb, q_d)):
                    t = persist.tile(
                        [P, QB], BF16, tag=f"{name}d{hp}_{b}", name=f"{name}d{hp}_{b}"
                    )
                    dst[hp].append(t)
                    ps = ps_u512.tile([P, QB], F32, tag="u512", name="u512")
                    for c in range(n_ct):
                        nc.tensor.matmul(
                            ps[:],
                            w_sb[c][:, hp * P : (hp + 1) * P],
                            xT_sb[c][:, b * QB : (b + 1) * QB],
                            start=(c == 0),
                            stop=(c == n_ct - 1),
                        )
                    nc.vector.tensor_copy(t[:], ps[:])

        def emit_attn(hp, qbs):
            for qb in qbs:
                yts = [
                    ps_u512.tile([D1, QB], F32, tag="u512", name="yt0"),
                    ps_u512.tile([D1, QB], F32, tag="u512", name="yt1"),
                ]
                n_kt = dpb * qb + dpb

                def emit_av(kt, pt):
                    # diagonal k-tiles only touch q-columns >= P*m
                    q0 = P * max(kt - dpb * qb, 0)
                    for i in range(2):
                        h = 2 * hp + i
                        nc.tensor.matmul(
                            yts[i][:, q0:QB],
                            v1_sb[kt][:, h * D1 : (h + 1) * D1],
                            pt[:, i * QB + q0 : (i + 1) * QB],
                            start=(kt == 0),
                            stop=(kt == n_kt - 1),
                            skip_group_check=True,
                        )

                pending = []
                for kt in range(n_kt):
                    m = kt - dpb * qb  # >=0: diagonal tile index
                    s0 = P * max(m, 0)  # diagonal tiles: only q-cols >= P*m used
                    # combined scores for both heads: [128 k, 1024]
                    st = ps_st2.tile([P, 2 * QB], F32, tag="st2", name="st2")
                    for i in range(2):
                        base = 64 * i
                        nc.tensor.matmul(
                            st[:, i * QB + s0 : (i + 1) * QB],
                            k_d[hp][kt // dpb][
                                base : base + 64, (kt % dpb) * P : (kt % dpb + 1) * P
                            ],
                            q_d[hp][qb][base : base + 64, s0:],
                            start=True,
                            stop=True,
                        )
                    pt = ppool.tile([P, 2 * QB], BF16, tag="pt", name="pt")
                    if m <= 0:
                        # one full-width exp covering both heads
                        nc.scalar.activation(
                            pt[:], st[:], mybir.ActivationFunctionType.Exp, scale=scale
                        )
                    else:
                        for i in range(2):
                            nc.scalar.activation(
                                pt[:, i * QB + s0 : (i + 1) * QB],
                                st[:, i * QB + s0 : (i + 1) * QB],
                                mybir.ActivationFunctionType.Exp,
                                scale=scale,
                            )
                    if m >= 0:
                        q0 = P * m
                        for i in range(2):
                            sl = pt[:, i * QB + q0 : i * QB + q0 + P]
                            nc.vector.tensor_mul(sl, sl, trimask[:])
                    # stagger: AV lags the scores by 2 k-tiles so the PE
                    # never queue-blocks on exp
                    pending.append((kt, pt))
                    if len(pending) > 2:
                        emit_av(*pending.pop(0))
                for item in pending:
                    emit_av(*item)

                # epilogue: one PSUM->SBUF copy per head, then DMA out the
                # unnormalized y (casting SWDGE) and the denominator row
                for i in range(2):
                    yt = yts[i]
                    ys = spool.tile([D1, QB], F32, tag="ys", name="ys")
                    nc.vector.tensor_copy(ys[:], yt[:])
                    nc.gpsimd.dma_start(
                        yT_sb[hp][64 * i : 64 * i + 64, qb * QB : (qb + 1) * QB],
                        ys[0:D, :],
                    )
                    s = sum_row(hp, qb, i)
                    nc.sync.dma_start(sums_d[s : s + 1, :], ys[D : D + 1, :])

        def emit_norm(hp):
            # reciprocal in two batches: all-but-last q-block's rows can be
            # reciprocal'd while the last q-block's attention still runs
            s0 = sum_row(hp, 0, 0)
            batches = []
            if n_qb > 1:
                batches.append((s0, 2 * (n_qb - 1)))
            batches.append((sum_row(hp, n_qb - 1, 0), 2))
            for bs, bn in batches:
                allsums = stage.tile([bn, QB], F32, tag="allsums", name="allsums")
                nc.sync.dma_start(allsums[:], sums_d[bs : bs + bn, :])
                allrec = stage.tile([bn, QB], F32, tag="allrec", name="allrec")
                nc.vector.reciprocal_approx_fast(allrec[:], allsums[:])
                nc.sync.dma_start(recips_d[bs : bs + bn, :], allrec[:])
            for qb in range(n_qb):
                bc = bcpool.tile([P, QB], F32, tag="bc", name="bc")
                for i in range(2):
                    s = sum_row(hp, qb, i)
                    nc.sync.dma_start(
                        bc[64 * i : 64 * i + 64, :],
                        recips_d[s : s + 1, :].to_broadcast((64, QB)),
                    )
                sl = yT_sb[hp][:, qb * QB : (qb + 1) * QB]
                nc.vector.tensor_mul(sl, sl, bc[:])

        # Emit qk for the next head pair between the second-to-last and last
        # q-block: the last (longest) q-block's attention is ACT-paced, and
        # its PSUM slot rotation then lets all qk matmuls fill PE stalls.
        emit_qk(0)
        emit_v(range(min(dpb, n_tt), n_tt))
        for hp in range(n_hp):
            emit_attn(hp, range(n_qb - 1))
            if hp + 1 < n_hp:
                emit_qk(hp + 1)
            emit_attn(hp, [n_qb - 1])
            emit_norm(hp)

        # ---- output projection (partial over this core's heads) ----
        for tt in range(n_tt):
            for cb in range(n_cb):
                ps = ps_u512.tile([P, QB], F32, tag="u512", name="u512")
                for m in range(n_mt):
                    nc.tensor.matmul(
                        ps[:],
                        yT_sb[m][:, tt * P : (tt + 1) * P],
                        wp_sb[m][:, cb * QB : (cb + 1) * QB],
                        start=(m == 0),
                        stop=(m == n_mt - 1),
                    )
                ost = stage.tile([P, QB], F32, tag="ostage", name="ostage")
                nc.vector.tensor_copy(ost[:], ps[:])
                nc.sync.dma_start(
                    out[tt * P : (tt + 1) * P, cb * QB : (cb + 1) * QB], ost[:]
                )

    nc.compile()
    return nc


_PROGRAM_CACHE = {}


def _get_program(C_eff):
    key = C_eff
    if key not in _PROGRAM_CACHE:
        _PROGRAM_CACHE[key] = build_kernel(T=T, C=C_eff, HL=HL, D=D, Cout=C)
    return _PROGRAM_CACHE[key]


def _make_in_maps(x, Wqkv, bqkv):
    """Shard + cast inputs for the 8 cores. Returns (in_maps, C_eff)."""
    if np.any(bqkv):
        # Fold the qkv bias in as an extra contraction row (x gains a ones
        # column), zero-padded up to a multiple of 128.
        C_eff = ((C + 1 + P - 1) // P) * P
        Waug = np.zeros((C_eff, 3 * C), dtype=np.float32)
        Waug[:C] = Wqkv
        Waug[C] = bqkv
    else:
        C_eff = C
        Waug = Wqkv

    masks = (np.arange(P)[:, None] <= np.arange(P)[None, :]).astype(NPBF16)
    in_maps = []
    for core in range(N_CORES):
        b, hh = divmod(core, N_CORES // B)
        xT = np.zeros((C_eff, T), dtype=np.float32)
        xT[:C] = x[b].T
        if C_eff > C:
            xT[C] = 1.0
        c0 = hh * CL
        in_maps.append(
            {
                "xT": xT.astype(NPBF16),
                "wq": np.ascontiguousarray(Waug[:, 0 * C + c0 : 0 * C + c0 + CL]).astype(NPBF16),
                "wk": np.ascontiguousarray(Waug[:, 1 * C + c0 : 1 * C + c0 + CL]).astype(NPBF16),
                "wv": np.ascontiguousarray(Waug[:, 2 * C + c0 : 2 * C + c0 + CL]).astype(NPBF16),
                "wp": None,  # filled below (depends only on hh)
                "masks": masks,
            }
        )
    return in_maps, C_eff


def _run(x, Wqkv, bqkv, Wproj, bproj, trace=False):
    from concourse.bass_utils import run_bass_kernel_spmd

    in_maps, C_eff = _make_in_maps(x, Wqkv, bqkv)
    wp_by_hh = [
        np.ascontiguousarray(Wproj[hh * CL : (hh + 1) * CL, :]).astype(NPBF16)
        for hh in range(N_CORES // B)
    ]
    for core in range(N_CORES):
        in_maps[core]["wp"] = wp_by_hh[core % (N_CORES // B)]

    nc = _get_program(C_eff)
    res = run_bass_kernel_spmd(
        nc, in_maps, core_ids=list(range(N_CORES)), trace=trace
    )

    halves = N_CORES // B
    y = np.empty((B, T, C), dtype=np.float32)
    for b in range(B):
        acc = res.results[b * halves]["out"].astype(np.float32)
        for hh in range(1, halves):
            acc = acc + res.results[b * halves + hh]["out"]
        y[b] = acc + bproj.astype(np.float32)
    return y, res


def kernel(x, Wqkv, bqkv, Wproj, bproj):
    y, _ = _run(
        np.asarray(x, dtype=np.float32),
        np.asarray(Wqkv, dtype=np.float32),
        np.asarray(bqkv, dtype=np.float32),
        np.asarray(Wproj, dtype=np.float32),
        np.asarray(bproj, dtype=np.float32),
        trace=False,
    )
    return y
